# revision 19
# baseline (speedup 1.0000x reference)
"""Causal self-attention (B=2, S=2048, D=1024, H=16) on 8 trn2 NeuronCores.

Sharding: head-parallel tensor parallelism. Core c owns heads {2c, 2c+1},
i.e. feature dims [128c, 128c+128) of q/k/v/y. Each core computes its
qT/kT/vT projections (transposed [dim, seq] layouts), causal attention in
score-transposed layout (S.T = k @ q.T blocks; softmax sums come free from
an appended ones column in v). Unnormalized y plus the per-(head, q)
softmax sums ride one AllToAll per batch, resharding to row-ownership:
core d owns rows [256d, 256d+256) of each batch. The receiver normalizes
(one batched reciprocal + selector-matmul partition-broadcast) and runs
the full output projection for its 512 rows. Host does layout prep
(transpose / slice / fp32r rounding) and output reassembly only.

All matmuls run in fp32r (8-bit exp / 11-bit mantissa) with fp32 PSUM.
"""

import numpy as np
import ml_dtypes

import concourse.bacc as bacc
import concourse.mybir as mybir
import concourse.tile as tile
from concourse.tile_rust import add_dep_helper
from concourse.bass_utils import run_bass_kernel_spmd

NC_CORES = 8
B, S, D, H = 2, 2048, 1024, 16
HD = D // H              # 64 head dim
BS = B * S               # 4096 flattened (b, q) rows
DC = D // NC_CORES       # 128 dims per core (2 heads)
NT = BS // 512           # 8 projection seq-tiles of 512
QS = 512                 # q-strip width
KB = 128                 # k-block height
NJ = S // QS             # 4 q-strips per batch
NKC = S // KB            # 16 k-chunks per batch
RB = S // NC_CORES       # 256 rows each core owns per batch
SCALE = 1.0 / 32.0       # 1/sqrt(D)

F32 = mybir.dt.float32
F32R = mybir.dt.float32r
BF16 = mybir.dt.bfloat16
AF = mybir.ActivationFunctionType

_CACHE = {}


def round_fp32r(a: np.ndarray) -> np.ndarray:
    """Round fp32 array to fp32r (8-bit exp, 11-bit mantissa), nearest-even."""
    u = np.ascontiguousarray(a, dtype=np.float32).view(np.uint32)
    low = u & np.uint32(0x00000FFF)
    base = u & np.uint32(0xFFFFF000)
    lsb = (u >> np.uint32(12)) & np.uint32(1)
    up = (low > 0x800) | ((low == 0x800) & (lsb == 1))
    return (base + np.where(up, np.uint32(0x1000), np.uint32(0)).astype(np.uint32)).view(np.float32)


def build_nc():
    nc = bacc.Bacc(num_devices=NC_CORES, num_swdge_queues=4)

    # ---- kernel I/O ----
    xT_in = nc.dram_tensor("xT", [NT, 8, 128, QS], BF16, kind="ExternalInput")  # [s, j, p, q]
    wq_in = nc.dram_tensor("wqT", [8, 128, DC], BF16, kind="ExternalInput")
    wk_in = nc.dram_tensor("wkT", [8, 128, DC], BF16, kind="ExternalInput")
    wv_in = nc.dram_tensor("wvT", [8, 128, DC], BF16, kind="ExternalInput")
    wp_in = nc.dram_tensor("wpT", [8, 128, 8, 128], F32R, kind="ExternalInput")  # [j, p, m, o]
    bq_in = nc.dram_tensor("bq", [DC], F32, kind="ExternalInput")
    bk_in = nc.dram_tensor("bk", [DC], F32, kind="ExternalInput")
    bv_in = nc.dram_tensor("bv", [DC], F32, kind="ExternalInput")
    bp_in = nc.dram_tensor("bpT", [128, 8], F32, kind="ExternalInput")
    mask_in = nc.dram_tensor("masks", [KB, 4, QS], F32, kind="ExternalInput")
    ident_in = nc.dram_tensor("ident", [128, 128], F32R, kind="ExternalInput")
    onesc_in = nc.dram_tensor("onescol", [128, NKC], F32R, kind="ExternalInput")
    sel_in = nc.dram_tensor("sel", [16, 8, 128], F32R, kind="ExternalInput")
    out_ext = nc.dram_tensor("outT", [D, 2 * RB], F32, kind="ExternalOutput")

    # A2A bounce buffers (internal DRAM), one pair per batch.
    # Layout [8 * 130, RB]: per dest chunk, 128 y-dim rows + 2 sums rows.
    a2a_src = [nc.dram_tensor(f"a2a_src{b}", [NC_CORES * 130, RB], F32R) for b in range(B)]
    a2a_dst = [nc.dram_tensor(f"a2a_dst{b}", [NC_CORES * 130, RB], F32R) for b in range(B)]

    xT_r = xT_in.ap().rearrange("s j p q -> s p j q")           # [8, 128, 8, 512]
    wq_r = wq_in.ap().rearrange("j p m -> p j m")               # [128, 8, 128]
    wk_r = wk_in.ap().rearrange("j p m -> p j m")
    wv_r = wv_in.ap().rearrange("j p m -> p j m")
    wp_r = wp_in.ap().rearrange("j p m o -> p j m o")           # [128, 8, 8, 128]
    out_r = out_ext.ap().rearrange("(m p) q -> p m q", p=128)   # [128, 8, 512]

    with tile.TileContext(nc) as tc:
        with (
            tc.tile_pool(name="consts", bufs=1) as consts,
            tc.tile_pool(name="wp", bufs=1) as wp_pool,
            tc.tile_pool(name="xt", bufs=2) as xt_pool,
            tc.tile_pool(name="seq", bufs=1) as seq_pool,
            tc.tile_pool(name="vt", bufs=3) as vt_pool,
            tc.tile_pool(name="es", bufs=3) as es_pool,
            tc.tile_pool(name="scr", bufs=2) as scr_pool,
            tc.tile_pool(name="nrm", bufs=2) as nrm_pool,
            tc.tile_pool(name="yf", bufs=1) as yf_pool,
            tc.tile_pool(name="ob", bufs=2) as ob_pool,
            tc.tile_pool(name="pmm", bufs=4, space="PSUM") as pmm,
            tc.tile_pool(name="py", bufs=2, space="PSUM") as py,
        ):
            # ---- constants needed early (weights for projections) ----
            wq_t = consts.tile([128, 8, DC], BF16)
            wk_t = consts.tile([128, 8, DC], BF16)
            wv_t = consts.tile([128, 8, DC], BF16)
            for j in range(8):
                eng = nc.sync if j % 2 == 0 else nc.scalar
                eng.dma_start(wq_t[:, j, :], wq_r[:, j, :])
            for j in range(8):
                eng = nc.sync if j % 2 == 0 else nc.scalar
                eng.dma_start(wk_t[:, j, :], wk_r[:, j, :])
                eng.dma_start(wv_t[:, j, :], wv_r[:, j, :])
            bq_t = consts.tile([DC, 1], F32)
            bk_t = consts.tile([DC, 1], F32)
            bv_t = consts.tile([DC, 1], F32)
            nc.sync.dma_start(bq_t[:], bq_in.ap().rearrange("(p one) -> p one", one=1))
            nc.sync.dma_start(bk_t[:], bk_in.ap().rearrange("(p one) -> p one", one=1))
            nc.sync.dma_start(bv_t[:], bv_in.ap().rearrange("(p one) -> p one", one=1))
            ident_t = consts.tile([128, 128], F32R)
            nc.gpsimd.dma_start(ident_t[:], ident_in[:])
            mask_t = consts.tile([KB, 4, QS], F32)
            nc.gpsimd.dma_start(mask_t[:], mask_in[:])
            sel_t = consts.tile([16, 8, 128], F32R)
            nc.gpsimd.dma_start(sel_t[:], sel_in[:])
            bp_t = consts.tile([128, 8], F32)
            nc.gpsimd.dma_start(bp_t[:], bp_in[:])

            # persistent per-core activations
            qt_sb = seq_pool.tile([DC, BS], F32R)       # q.T  [dims, (b q)]
            kt_sb = seq_pool.tile([DC, BS], F32R)       # k.T
            # v in natural [k, dim] layout per batch, 16 chunks of 128 k each;
            # chunk layout (2, 66): g = [head dims(64), ones, pad] so the AV
            # lhsT slices [0:65] / [66:131] both put softmax sums at row 64.
            v2 = seq_pool.tile([128, B, NKC, 132], F32R)

            # ones columns of v2 (written once, from host constant)
            for b in range(B):
                nc.gpsimd.dma_start(
                    v2[:, b, :, 64:65],
                    onesc_in.ap().rearrange("p (t one) -> p t one", one=1),
                )
                nc.gpsimd.dma_start(
                    v2[:, b, :, 130:131],
                    onesc_in.ap().rearrange("p (t one) -> p t one", one=1),
                )

            # ---- phase 1: projections (q.T, k.T, v.T -> v) ----
            def proj_tile(s):
                b = s // NJ
                xt_s = xt_pool.tile([128, 8, QS], BF16)
                for j in range(8):
                    eng = (nc.sync, nc.scalar, nc.gpsimd, nc.gpsimd)[j % 4]
                    eng.dma_start(xt_s[:, j, :], xT_r[s, :, j, :])

                vt_s = vt_pool.tile([DC, QS], F32R)
                for name, w_t, b_t, dst in (
                    ("q", wq_t, bq_t, qt_sb),
                    ("k", wk_t, bk_t, kt_sb),
                    ("v", wv_t, bv_t, None),
                ):
                    acc = pmm.tile([DC, QS], F32, tag="mm")
                    for j in range(8):
                        nc.tensor.matmul(
                            acc[:],
                            w_t[:, j, :],
                            xt_s[:, j, :],
                            start=(j == 0),
                            stop=(j == 7),
                        )
                    out_slice = dst[:, s * QS:(s + 1) * QS] if dst is not None else vt_s[:]
                    nc.scalar.activation(out_slice, acc[:], AF.Identity, bias=b_t[:], scale=1.0)

                # transpose v.T strip into natural-layout v2 chunks
                for tloc in range(4):
                    kc = (s % NJ) * 4 + tloc       # k-chunk index within batch
                    vp = pmm.tile([128, 128], F32R, tag="mm")
                    nc.tensor.transpose(vp[:], vt_s[:, tloc * 128:(tloc + 1) * 128], ident_t[:])
                    dst_ap = v2[:, b, kc, :].rearrange("p (g r) -> p g r", g=2)
                    nc.vector.tensor_copy(
                        dst_ap[:, :, 0:64],
                        vp[:, :].rearrange("p (g r) -> p g r", g=2),
                    )

            # ---- phase 2+3: causal attention + per-batch A2A ----
            # Per-strip results stream straight into the A2A source buffer
            # (q-strip j covers dest blocks 2j, 2j+1 of this batch).
            def attention_strip(b, j):
                    src_r = a2a_src[b].ap().rearrange("(d r) q -> r d q", r=130)
                    qcol = b * S + j * QS
                    for hh in range(2):
                        hr = slice(hh * HD, (hh + 1) * HD)   # head rows in qt/kt
                        ya = py.tile([65, QS], F32, tag="ya")
                        nblk = 4 * j + 4
                        for t in range(nblk):
                            sp = pmm.tile([KB, QS], F32, tag="mm")
                            nc.tensor.matmul(
                                sp[:],
                                kt_sb[hr, b * S + t * KB: b * S + (t + 1) * KB],
                                qt_sb[hr, qcol: qcol + QS],
                            )
                            es = es_pool.tile([KB, QS], F32R)
                            mi = t - 4 * j
                            if mi < 0:
                                nc.scalar.activation(es[:], sp[:], AF.Exp, scale=SCALE)
                            else:
                                sc = scr_pool.tile([KB, QS], F32)
                                nc.scalar.activation(sc[:], sp[:], AF.Exp, scale=SCALE)
                                nc.vector.tensor_mul(es[:], sc[:], mask_t[:, mi, :])
                            lo = 0 if hh == 0 else 66
                            nc.tensor.matmul(
                                ya[:],
                                v2[:, b, t, lo:lo + 65],
                                es[:],
                                start=(t == 0),
                                stop=(t == nblk - 1),
                            )
                        # extract unnormalized dims + sums (row 64), stream to
                        # the A2A source in DRAM
                        ystrip = scr_pool.tile([64, QS], F32R, tag="ystrip", bufs=3)
                        nc.vector.tensor_copy(ystrip[:], ya[0:64, :])
                        srow = scr_pool.tile([1, QS], F32R, tag="srow", bufs=4)
                        nc.scalar.activation(srow[:], ya[64:65, :], AF.Copy)
                        nc.sync.dma_start(
                            src_r[hh * 64:(hh + 1) * 64, 2 * j:2 * j + 2, :],
                            ystrip[:].rearrange("p (d q) -> p d q", q=RB),
                        )
                        nc.sync.dma_start(
                            src_r[128 + hh:129 + hh, 2 * j:2 * j + 2, :],
                            srow[:].rearrange("p (d q) -> p d q", q=RB),
                        )

            def issue_a2a(b):
                return nc.gpsimd.collective_compute(
                    "AllToAll",
                    mybir.AluOpType.bypass,
                    replica_groups=[list(range(NC_CORES))],
                    ins=[a2a_src[b].ap().opt()],
                    outs=[a2a_dst[b].ap().opt()],
                )

            def rn_pre(b):
                """Load A2A result, batch-reciprocal the sums (DVE/DMA only)."""
                dst_r = a2a_dst[b].ap().rearrange("(d r) q -> d r q", r=130)
                yfull = yf_pool.tile([128, 8, RB], F32R, tag="yfull", bufs=2)
                nc.sync.dma_start(
                    yfull[:],
                    a2a_dst[b].ap().rearrange("(d r) q -> r d q", r=130)[0:128],
                )
                sfull = nrm_pool.tile([16, RB], F32R, tag="sf")
                nc.sync.dma_start(sfull[0:8, :], dst_r[:, 128, :])
                nc.sync.dma_start(sfull[8:16, :], dst_r[:, 129, :])
                rec_f = nrm_pool.tile([16, RB], F32, tag="recf")
                nc.vector.reciprocal(rec_f[:], sfull[:])
                rec_r = nrm_pool.tile([16, RB], F32R, tag="recr")
                nc.vector.tensor_copy(rec_r[:], rec_f[:])
                return yfull, rec_r

            def rn_mm(b, yfull, rec_r):
                """Broadcast-normalize (PE + DVE)."""
                ynj = yf_pool.tile([128, 8, RB], F32R, tag="ynj", bufs=1)
                for j in range(8):
                    bc = pmm.tile([128, RB], F32, tag="mm")
                    nc.tensor.matmul(bc[:], sel_t[:, j, :], rec_r[:])
                    nc.vector.tensor_mul(ynj[:, j, :], yfull[:, j, :], bc[:])
                return ynj

            def outproj(b, ynj):
                for m in range(8):
                    oacc = pmm.tile([128, RB], F32, tag="mm")
                    for j in range(8):
                        nc.tensor.matmul(
                            oacc[:],
                            wp_t[:, j, m, :],
                            ynj[:, j, :],
                            start=(j == 0),
                            stop=(j == 7),
                        )
                    osb = ob_pool.tile([128, RB], F32)
                    nc.scalar.activation(osb[:], oacc[:], AF.Identity,
                                         bias=bp_t[:, m:m + 1], scale=1.0)
                    nc.sync.dma_start(out_r[:, m, b * RB:(b + 1) * RB], osb[:])

            # interleave: attention strip (0, j) right after proj tile s=j
            for s in range(NJ):
                proj_tile(s)
                attention_strip(0, s)
            cc0 = issue_a2a(0)

            # wp loads mid-stream: only needed for out-proj. Pin them behind
            # the first collective so the scheduler can't hoist them into the
            # projection phase's DMA window.
            wp_t = wp_pool.tile([128, 8, 8, 128], F32R)
            for j in range(8):
                wdma = nc.gpsimd.dma_start(wp_t[:, j, :, :], wp_r[:, j, :, :])
                add_dep_helper(wdma.ins, cc0.ins, sync=False,
                               reason="wp loads after a2a0 issue")

            for s in range(NJ, NT):
                proj_tile(s)
                attention_strip(1, s - NJ)
            issue_a2a(1)

            rn0 = rn_pre(0)
            rn1 = rn_pre(1)
            outproj(0, rn_mm(0, *rn0))
            outproj(1, rn_mm(1, *rn1))

    nc.compile()
    return nc


def _host_inputs(x, Wq, bq, Wk, bk, Wv, bv, Wp, bp):
    """Build the 8 per-core input maps (host does layout prep only)."""
    x = np.asarray(x, dtype=np.float32).reshape(BS, D)
    # xT pre-tiled [s, j, p, q]: chunk (s, j) = x.T[128j:128j+128, 512s:512s+512]
    xT = np.ascontiguousarray(
        x.T.reshape(8, 128, NT, QS).transpose(2, 0, 1, 3)
    ).astype(ml_dtypes.bfloat16)
    # WpT pre-tiled [j, p, m, o]: Wp.T[128j+p, 128m+o]
    WpT = round_fp32r(
        np.ascontiguousarray(
            np.asarray(Wp, np.float32).T.reshape(8, 128, 8, 128)
        )
    )
    bpT = np.ascontiguousarray(np.asarray(bp, np.float32).reshape(8, 128).T)

    # causal 0/1 masks for diagonal blocks: mask[mi][kk, qq] = qq >= kk + 128*mi
    kk = np.arange(KB)[:, None]
    qq = np.arange(QS)[None, :]
    masks = np.stack([(qq >= kk + 128 * mi).astype(np.float32) for mi in range(4)], axis=1)

    ident = round_fp32r(np.eye(128, dtype=np.float32))
    onescol = np.ones((128, NKC), dtype=np.float32)

    # selector for receiver-side normalize: bc_j = sel[:, j, :].T @ recip
    # bc_j rows 0..63 take head 2j (= sfull row j), rows 64..127 head 2j+1
    # (= sfull row 8+j).
    sel = np.zeros((16, 8, 128), dtype=np.float32)
    for j in range(8):
        sel[j, j, 0:64] = 1.0
        sel[8 + j, j, 64:128] = 1.0

    in_maps = []
    for c in range(NC_CORES):
        dsl = slice(DC * c, DC * (c + 1))
        in_maps.append({
            "xT": xT,
            "wqT": np.ascontiguousarray(
                np.asarray(Wq, np.float32)[dsl, :].T.reshape(8, 128, DC)).astype(ml_dtypes.bfloat16),
            "wkT": np.ascontiguousarray(
                np.asarray(Wk, np.float32)[dsl, :].T.reshape(8, 128, DC)).astype(ml_dtypes.bfloat16),
            "wvT": np.ascontiguousarray(
                np.asarray(Wv, np.float32)[dsl, :].T.reshape(8, 128, DC)).astype(ml_dtypes.bfloat16),
            "wpT": WpT,
            "bq": np.asarray(bq, np.float32)[dsl].copy(),
            "bk": np.asarray(bk, np.float32)[dsl].copy(),
            "bv": np.asarray(bv, np.float32)[dsl].copy(),
            "bpT": bpT,
            "masks": masks,
            "ident": ident,
            "onescol": onescol,
            "sel": sel,
        })
    return in_maps


def _assemble(results):
    """results[c]["outT"] is [D, 512]: cols 0:256 = batch-0 rows
    [256c, 256c+256), cols 256:512 = batch-1 rows [256c, 256c+256)."""
    flat = np.empty((BS, D), dtype=np.float32)
    for c in range(NC_CORES):
        o = results[c]["outT"]
        flat[RB * c: RB * (c + 1), :] = o[:, 0:RB].T
        flat[S + RB * c: S + RB * (c + 1), :] = o[:, RB:2 * RB].T
    return flat.reshape(B, S, D)


def run(inputs, trace=False, **kw):
    if "nc" not in _CACHE:
        _CACHE["nc"] = build_nc()
    nc = _CACHE["nc"]
    in_maps = _host_inputs(**inputs)
    res = run_bass_kernel_spmd(
        nc, in_maps, core_ids=list(range(NC_CORES)), trace=trace, **kw
    )
    return _assemble(res.results), res


def kernel(**inputs) -> np.ndarray:
    out, _ = run(inputs, trace=False)
    return out


# revision 20
# speedup vs baseline: 1.0517x; 1.0517x over previous
"""Causal self-attention (B=2, S=2048, D=1024, H=16) on 8 trn2 NeuronCores.

Sharding: head-parallel tensor parallelism. Core c owns heads {2c, 2c+1},
i.e. feature dims [128c, 128c+128) of q/k/v/y. Each core computes its
qT/kT/vT projections (transposed [dim, seq] layouts), causal attention in
score-transposed layout (S.T = k @ q.T blocks; softmax sums come free from
an appended ones column in v). Unnormalized y plus the per-(head, q)
softmax sums ride one AllToAll per batch, resharding to row-ownership:
core d owns rows [256d, 256d+256) of each batch. The receiver normalizes
(one batched reciprocal + selector-matmul partition-broadcast) and runs
the full output projection for its 512 rows. Host does layout prep
(transpose / slice / fp32r rounding) and output reassembly only.

All matmuls run in fp32r (8-bit exp / 11-bit mantissa) with fp32 PSUM.
"""

import numpy as np
import ml_dtypes

import concourse.bacc as bacc
import concourse.mybir as mybir
import concourse.tile as tile
from concourse.tile_rust import add_dep_helper
from concourse.bass_utils import run_bass_kernel_spmd

NC_CORES = 8
B, S, D, H = 2, 2048, 1024, 16
HD = D // H              # 64 head dim
BS = B * S               # 4096 flattened (b, q) rows
DC = D // NC_CORES       # 128 dims per core (2 heads)
NT = BS // 512           # 8 projection seq-tiles of 512
QS = 512                 # q-strip width
KB = 128                 # k-block height
NJ = S // QS             # 4 q-strips per batch
NKC = S // KB            # 16 k-chunks per batch
RB = S // NC_CORES       # 256 rows each core owns per batch
SCALE = 1.0 / 32.0       # 1/sqrt(D)

F32 = mybir.dt.float32
F32R = mybir.dt.float32r
BF16 = mybir.dt.bfloat16
AF = mybir.ActivationFunctionType

_CACHE = {}


def round_fp32r(a: np.ndarray) -> np.ndarray:
    """Round fp32 array to fp32r (8-bit exp, 11-bit mantissa), nearest-even."""
    u = np.ascontiguousarray(a, dtype=np.float32).view(np.uint32)
    low = u & np.uint32(0x00000FFF)
    base = u & np.uint32(0xFFFFF000)
    lsb = (u >> np.uint32(12)) & np.uint32(1)
    up = (low > 0x800) | ((low == 0x800) & (lsb == 1))
    return (base + np.where(up, np.uint32(0x1000), np.uint32(0)).astype(np.uint32)).view(np.float32)


def build_nc():
    nc = bacc.Bacc(num_devices=NC_CORES, num_swdge_queues=4)

    # ---- kernel I/O ----
    xT_in = nc.dram_tensor("xT", [NT, 8, 128, QS], BF16, kind="ExternalInput")  # [s, j, p, q]
    wq_in = nc.dram_tensor("wqT", [8, 128, DC], BF16, kind="ExternalInput")
    wk_in = nc.dram_tensor("wkT", [8, 128, DC], BF16, kind="ExternalInput")
    wv_in = nc.dram_tensor("wvT", [8, 128, DC], BF16, kind="ExternalInput")
    wp_in = nc.dram_tensor("wpT", [8, 128, 8, 128], F32R, kind="ExternalInput")  # [j, p, m, o]
    bq_in = nc.dram_tensor("bq", [DC], F32, kind="ExternalInput")
    bk_in = nc.dram_tensor("bk", [DC], F32, kind="ExternalInput")
    bv_in = nc.dram_tensor("bv", [DC], F32, kind="ExternalInput")
    bp_in = nc.dram_tensor("bpT", [128, 8], F32, kind="ExternalInput")
    mask_in = nc.dram_tensor("masks", [KB, 4, QS], F32, kind="ExternalInput")
    ident_in = nc.dram_tensor("ident", [128, 128], F32R, kind="ExternalInput")
    onesc_in = nc.dram_tensor("onescol", [128, NKC], F32R, kind="ExternalInput")
    sel_in = nc.dram_tensor("sel", [16, 8, 128], F32R, kind="ExternalInput")
    out_ext = nc.dram_tensor("outT", [D, 2 * RB], F32, kind="ExternalOutput")

    # A2A bounce buffers (internal DRAM), one pair per batch.
    # Layout [8 * 130, RB]: per dest chunk, 128 y-dim rows + 2 sums rows.
    a2a_src = [nc.dram_tensor(f"a2a_src{b}", [NC_CORES * 130, RB], F32R) for b in range(B)]
    a2a_dst = [nc.dram_tensor(f"a2a_dst{b}", [NC_CORES * 130, RB], F32R) for b in range(B)]

    xT_r = xT_in.ap().rearrange("s j p q -> s p j q")           # [8, 128, 8, 512]
    wq_r = wq_in.ap().rearrange("j p m -> p j m")               # [128, 8, 128]
    wk_r = wk_in.ap().rearrange("j p m -> p j m")
    wv_r = wv_in.ap().rearrange("j p m -> p j m")
    wp_r = wp_in.ap().rearrange("j p m o -> p j m o")           # [128, 8, 8, 128]
    out_r = out_ext.ap().rearrange("(m p) q -> p m q", p=128)   # [128, 8, 512]

    with tile.TileContext(nc) as tc:
        with (
            tc.tile_pool(name="consts", bufs=1) as consts,
            tc.tile_pool(name="wp", bufs=1) as wp_pool,
            tc.tile_pool(name="xt", bufs=3) as xt_pool,
            tc.tile_pool(name="seq", bufs=1) as seq_pool,
            tc.tile_pool(name="vt", bufs=3) as vt_pool,
            tc.tile_pool(name="es", bufs=3) as es_pool,
            tc.tile_pool(name="scr", bufs=2) as scr_pool,
            tc.tile_pool(name="nrm", bufs=2) as nrm_pool,
            tc.tile_pool(name="yf", bufs=1) as yf_pool,
            tc.tile_pool(name="ob", bufs=2) as ob_pool,
            tc.tile_pool(name="pmm", bufs=4, space="PSUM") as pmm,
            tc.tile_pool(name="py", bufs=2, space="PSUM") as py,
        ):
            # ---- constants needed early (weights for projections) ----
            wq_t = consts.tile([128, 8, DC], BF16)
            wk_t = consts.tile([128, 8, DC], BF16)
            wv_t = consts.tile([128, 8, DC], BF16)
            for j in range(8):
                eng = nc.sync if j % 2 == 0 else nc.scalar
                eng.dma_start(wq_t[:, j, :], wq_r[:, j, :])
            for j in range(8):
                eng = nc.sync if j % 2 == 0 else nc.scalar
                eng.dma_start(wk_t[:, j, :], wk_r[:, j, :])
                eng.dma_start(wv_t[:, j, :], wv_r[:, j, :])
            bq_t = consts.tile([DC, 1], F32)
            bk_t = consts.tile([DC, 1], F32)
            bv_t = consts.tile([DC, 1], F32)
            nc.sync.dma_start(bq_t[:], bq_in.ap().rearrange("(p one) -> p one", one=1))
            nc.sync.dma_start(bk_t[:], bk_in.ap().rearrange("(p one) -> p one", one=1))
            nc.sync.dma_start(bv_t[:], bv_in.ap().rearrange("(p one) -> p one", one=1))
            ident_t = consts.tile([128, 128], F32R)
            nc.gpsimd.dma_start(ident_t[:], ident_in[:])
            mask_t = consts.tile([KB, 4, QS], F32)
            nc.gpsimd.dma_start(mask_t[:], mask_in[:])
            sel_t = consts.tile([16, 8, 128], F32R)
            nc.gpsimd.dma_start(sel_t[:], sel_in[:])
            bp_t = consts.tile([128, 8], F32)
            nc.gpsimd.dma_start(bp_t[:], bp_in[:])

            # persistent per-core activations
            qt_sb = seq_pool.tile([DC, BS], F32R)       # q.T  [dims, (b q)]
            kt_sb = seq_pool.tile([DC, BS], F32R)       # k.T
            # v in natural [k, dim] layout per batch, 16 chunks of 128 k each;
            # chunk layout (2, 66): g = [head dims(64), ones, pad] so the AV
            # lhsT slices [0:65] / [66:131] both put softmax sums at row 64.
            v2 = seq_pool.tile([128, B, NKC, 132], F32R)

            # ones columns of v2 (written once, from host constant)
            for b in range(B):
                nc.gpsimd.dma_start(
                    v2[:, b, :, 64:65],
                    onesc_in.ap().rearrange("p (t one) -> p t one", one=1),
                )
                nc.gpsimd.dma_start(
                    v2[:, b, :, 130:131],
                    onesc_in.ap().rearrange("p (t one) -> p t one", one=1),
                )

            # ---- phase 1: projections (q.T, k.T, v.T -> v) ----
            def proj_tile(s):
                b = s // NJ
                xt_s = xt_pool.tile([128, 8, QS], BF16)
                for j in range(8):
                    # batch-0 tiles ride sync+gpsimd; batch-1 tiles must stay
                    # off the Pool queue (blocked behind the A2A#0 collective)
                    if s < NJ:
                        eng = (nc.sync, nc.gpsimd)[j % 2]
                    else:
                        eng = (nc.sync, nc.scalar)[j % 2]
                    eng.dma_start(xt_s[:, j, :], xT_r[s, :, j, :])

                vt_s = vt_pool.tile([DC, QS], F32R)
                for name, w_t, b_t, dst in (
                    ("q", wq_t, bq_t, qt_sb),
                    ("k", wk_t, bk_t, kt_sb),
                    ("v", wv_t, bv_t, None),
                ):
                    acc = pmm.tile([DC, QS], F32, tag="mm")
                    for j in range(8):
                        nc.tensor.matmul(
                            acc[:],
                            w_t[:, j, :],
                            xt_s[:, j, :],
                            start=(j == 0),
                            stop=(j == 7),
                        )
                    out_slice = dst[:, s * QS:(s + 1) * QS] if dst is not None else vt_s[:]
                    nc.scalar.activation(out_slice, acc[:], AF.Identity, bias=b_t[:], scale=1.0)

                # transpose v.T strip into natural-layout v2 chunks
                for tloc in range(4):
                    kc = (s % NJ) * 4 + tloc       # k-chunk index within batch
                    vp = pmm.tile([128, 128], F32R, tag="mm")
                    nc.tensor.transpose(vp[:], vt_s[:, tloc * 128:(tloc + 1) * 128], ident_t[:])
                    dst_ap = v2[:, b, kc, :].rearrange("p (g r) -> p g r", g=2)
                    nc.vector.tensor_copy(
                        dst_ap[:, :, 0:64],
                        vp[:, :].rearrange("p (g r) -> p g r", g=2),
                    )

            # ---- phase 2+3: causal attention + per-batch A2A ----
            # Per-strip results stream straight into the A2A source buffer
            # (q-strip j covers dest blocks 2j, 2j+1 of this batch).
            def attention_strip(b, j):
                    src_r = a2a_src[b].ap().rearrange("(d r) q -> r d q", r=130)
                    qcol = b * S + j * QS
                    for hh in range(2):
                        hr = slice(hh * HD, (hh + 1) * HD)   # head rows in qt/kt
                        ya = py.tile([65, QS], F32, tag="ya")
                        nblk = 4 * j + 4
                        for t in range(nblk):
                            sp = pmm.tile([KB, QS], F32, tag="mm")
                            nc.tensor.matmul(
                                sp[:],
                                kt_sb[hr, b * S + t * KB: b * S + (t + 1) * KB],
                                qt_sb[hr, qcol: qcol + QS],
                            )
                            es = es_pool.tile([KB, QS], F32R)
                            mi = t - 4 * j
                            if mi < 0:
                                nc.scalar.activation(es[:], sp[:], AF.Exp, scale=SCALE)
                            else:
                                sc = scr_pool.tile([KB, QS], F32)
                                nc.scalar.activation(sc[:], sp[:], AF.Exp, scale=SCALE)
                                nc.vector.tensor_mul(es[:], sc[:], mask_t[:, mi, :])
                            lo = 0 if hh == 0 else 66
                            nc.tensor.matmul(
                                ya[:],
                                v2[:, b, t, lo:lo + 65],
                                es[:],
                                start=(t == 0),
                                stop=(t == nblk - 1),
                            )
                        # extract unnormalized dims + sums (row 64), stream to
                        # the A2A source in DRAM
                        ystrip = scr_pool.tile([64, QS], F32R, tag="ystrip", bufs=3)
                        nc.vector.tensor_copy(ystrip[:], ya[0:64, :])
                        srow = scr_pool.tile([1, QS], F32R, tag="srow", bufs=4)
                        nc.scalar.activation(srow[:], ya[64:65, :], AF.Copy)
                        nc.sync.dma_start(
                            src_r[hh * 64:(hh + 1) * 64, 2 * j:2 * j + 2, :],
                            ystrip[:].rearrange("p (d q) -> p d q", q=RB),
                        )
                        nc.sync.dma_start(
                            src_r[128 + hh:129 + hh, 2 * j:2 * j + 2, :],
                            srow[:].rearrange("p (d q) -> p d q", q=RB),
                        )

            def issue_a2a(b):
                return nc.gpsimd.collective_compute(
                    "AllToAll",
                    mybir.AluOpType.bypass,
                    replica_groups=[list(range(NC_CORES))],
                    ins=[a2a_src[b].ap().opt()],
                    outs=[a2a_dst[b].ap().opt()],
                )

            def rn_pre(b):
                """Load A2A result, batch-reciprocal the sums (DVE/DMA only)."""
                dst_r = a2a_dst[b].ap().rearrange("(d r) q -> d r q", r=130)
                yfull = yf_pool.tile([128, 8, RB], F32R, tag="yfull", bufs=2)
                nc.sync.dma_start(
                    yfull[:],
                    a2a_dst[b].ap().rearrange("(d r) q -> r d q", r=130)[0:128],
                )
                sfull = nrm_pool.tile([16, RB], F32R, tag="sf")
                nc.sync.dma_start(sfull[0:8, :], dst_r[:, 128, :])
                nc.sync.dma_start(sfull[8:16, :], dst_r[:, 129, :])
                rec_f = nrm_pool.tile([16, RB], F32, tag="recf")
                nc.vector.reciprocal(rec_f[:], sfull[:])
                rec_r = nrm_pool.tile([16, RB], F32R, tag="recr")
                nc.vector.tensor_copy(rec_r[:], rec_f[:])
                return yfull, rec_r

            def rn_mm(b, yfull, rec_r):
                """Broadcast-normalize (PE + DVE)."""
                ynj = yf_pool.tile([128, 8, RB], F32R, tag="ynj", bufs=1)
                for j in range(8):
                    bc = pmm.tile([128, RB], F32, tag="mm")
                    nc.tensor.matmul(bc[:], sel_t[:, j, :], rec_r[:])
                    nc.vector.tensor_mul(ynj[:, j, :], yfull[:, j, :], bc[:])
                return ynj

            def outproj(b, ynj):
                for m in range(8):
                    oacc = pmm.tile([128, RB], F32, tag="mm")
                    for j in range(8):
                        nc.tensor.matmul(
                            oacc[:],
                            wp_t[:, j, m, :],
                            ynj[:, j, :],
                            start=(j == 0),
                            stop=(j == 7),
                        )
                    osb = ob_pool.tile([128, RB], F32)
                    nc.scalar.activation(osb[:], oacc[:], AF.Identity,
                                         bias=bp_t[:, m:m + 1], scale=1.0)
                    nc.sync.dma_start(out_r[:, m, b * RB:(b + 1) * RB], osb[:])

            # interleave: attention strip (0, j) right after proj tile s=j
            for s in range(NJ):
                proj_tile(s)
                attention_strip(0, s)
            cc0 = issue_a2a(0)

            # wp loads mid-stream: only needed for out-proj. Pin them behind
            # the first collective so the scheduler can't hoist them into the
            # projection phase's DMA window.
            wp_t = wp_pool.tile([128, 8, 8, 128], F32R)
            for j in range(8):
                wdma = nc.gpsimd.dma_start(wp_t[:, j, :, :], wp_r[:, j, :, :])
                add_dep_helper(wdma.ins, cc0.ins, sync=False,
                               reason="wp loads after a2a0 issue")

            for s in range(NJ, NT):
                proj_tile(s)
                attention_strip(1, s - NJ)
            issue_a2a(1)

            rn0 = rn_pre(0)
            rn1 = rn_pre(1)
            outproj(0, rn_mm(0, *rn0))
            outproj(1, rn_mm(1, *rn1))

    nc.compile()
    return nc


def _host_inputs(x, Wq, bq, Wk, bk, Wv, bv, Wp, bp):
    """Build the 8 per-core input maps (host does layout prep only)."""
    x = np.asarray(x, dtype=np.float32).reshape(BS, D)
    # xT pre-tiled [s, j, p, q]: chunk (s, j) = x.T[128j:128j+128, 512s:512s+512]
    xT = np.ascontiguousarray(
        x.T.reshape(8, 128, NT, QS).transpose(2, 0, 1, 3)
    ).astype(ml_dtypes.bfloat16)
    # WpT pre-tiled [j, p, m, o]: Wp.T[128j+p, 128m+o]
    WpT = round_fp32r(
        np.ascontiguousarray(
            np.asarray(Wp, np.float32).T.reshape(8, 128, 8, 128)
        )
    )
    bpT = np.ascontiguousarray(np.asarray(bp, np.float32).reshape(8, 128).T)

    # causal 0/1 masks for diagonal blocks: mask[mi][kk, qq] = qq >= kk + 128*mi
    kk = np.arange(KB)[:, None]
    qq = np.arange(QS)[None, :]
    masks = np.stack([(qq >= kk + 128 * mi).astype(np.float32) for mi in range(4)], axis=1)

    ident = round_fp32r(np.eye(128, dtype=np.float32))
    onescol = np.ones((128, NKC), dtype=np.float32)

    # selector for receiver-side normalize: bc_j = sel[:, j, :].T @ recip
    # bc_j rows 0..63 take head 2j (= sfull row j), rows 64..127 head 2j+1
    # (= sfull row 8+j).
    sel = np.zeros((16, 8, 128), dtype=np.float32)
    for j in range(8):
        sel[j, j, 0:64] = 1.0
        sel[8 + j, j, 64:128] = 1.0

    in_maps = []
    for c in range(NC_CORES):
        dsl = slice(DC * c, DC * (c + 1))
        in_maps.append({
            "xT": xT,
            "wqT": np.ascontiguousarray(
                np.asarray(Wq, np.float32)[dsl, :].T.reshape(8, 128, DC)).astype(ml_dtypes.bfloat16),
            "wkT": np.ascontiguousarray(
                np.asarray(Wk, np.float32)[dsl, :].T.reshape(8, 128, DC)).astype(ml_dtypes.bfloat16),
            "wvT": np.ascontiguousarray(
                np.asarray(Wv, np.float32)[dsl, :].T.reshape(8, 128, DC)).astype(ml_dtypes.bfloat16),
            "wpT": WpT,
            "bq": np.asarray(bq, np.float32)[dsl].copy(),
            "bk": np.asarray(bk, np.float32)[dsl].copy(),
            "bv": np.asarray(bv, np.float32)[dsl].copy(),
            "bpT": bpT,
            "masks": masks,
            "ident": ident,
            "onescol": onescol,
            "sel": sel,
        })
    return in_maps


def _assemble(results):
    """results[c]["outT"] is [D, 512]: cols 0:256 = batch-0 rows
    [256c, 256c+256), cols 256:512 = batch-1 rows [256c, 256c+256)."""
    flat = np.empty((BS, D), dtype=np.float32)
    for c in range(NC_CORES):
        o = results[c]["outT"]
        flat[RB * c: RB * (c + 1), :] = o[:, 0:RB].T
        flat[S + RB * c: S + RB * (c + 1), :] = o[:, RB:2 * RB].T
    return flat.reshape(B, S, D)


def run(inputs, trace=False, **kw):
    if "nc" not in _CACHE:
        _CACHE["nc"] = build_nc()
    nc = _CACHE["nc"]
    in_maps = _host_inputs(**inputs)
    res = run_bass_kernel_spmd(
        nc, in_maps, core_ids=list(range(NC_CORES)), trace=trace, **kw
    )
    return _assemble(res.results), res


def kernel(**inputs) -> np.ndarray:
    out, _ = run(inputs, trace=False)
    return out
